# revision 3
# baseline (speedup 1.0000x reference)
"""Multi-head attention (B=4, S=2048, D=512, H=8) on 8 trn2 NeuronCores.

Sharding: core c handles batch b = c//2 and head-group g = c%2 (4 heads,
256 of the 512 model dims). Each core computes its 4 heads' attention and
a partial out-projection [2048, 512]; the host sums the two partials per
batch and adds the output bias.

Device kernel per core (all matmuls bf16 -> f32 PSUM):
  1. QKV projections from pre-transposed xT [512, 2048]:
       Q^T/K^T [128, 2048] per head-pair (wq, bq pre-scaled by 1/8 on host)
       V [128, 260] per seq-tile of 128, with a per-head all-ones column
       (injected via the bias) so the P@V matmul also produces softmax
       row-sums.
  2. Per head, flash-style: S^T tile [128, 1024] = K_h^T . Q_h (K=64),
     exp on ScalarE (PSUM -> SBUF bf16, double-buffered, software
     pipelined), P^T accumulated into O^T [65, 512] over 16 k-tiles.
  3. Normalize: reciprocal of the row-sum row, broadcast via a K=1 ones
     matmul, multiply into the stacked O^T [128, 2048] per pair.
  4. Out-projection: out^T partial = sum_p OT[p]^T(chunk) @ wo[p].
No max-subtraction in softmax: scores are O(1) by construction, exp is
safe, and the reference softmax is shift-invariant.
"""

import numpy as np
import ml_dtypes

import concourse.bacc as bacc
import concourse.mybir as mybir
from concourse.tile import TileContext
from concourse.bass_utils import run_bass_kernel_spmd

BF16 = mybir.dt.bfloat16
F32 = mybir.dt.float32
AF = mybir.ActivationFunctionType
ALU = mybir.AluOpType

B, S, D = 4, 2048, 512
H_CORE, HD = 4, 64          # heads per core, head dim
DHC = H_CORE * HD           # 256 dims per core
VW = H_CORE * (HD + 1)      # 260: V augmented with one ones-column per head
N_CORES = 8

_CACHE = {}


def build_nc():
    nc = bacc.Bacc("TRN2", target_bir_lowering=False, debug=False,
                   num_devices=N_CORES)

    xT_d = nc.declare_dram_parameter("xT", [D, S], BF16, isOutput=False)
    wq_d = nc.declare_dram_parameter("wq", [D, DHC], BF16, isOutput=False)
    wk_d = nc.declare_dram_parameter("wk", [D, DHC], BF16, isOutput=False)
    wv_d = nc.declare_dram_parameter("wv", [D, VW], BF16, isOutput=False)
    wo_d = nc.declare_dram_parameter("wo", [DHC, D], BF16, isOutput=False)
    bq_d = nc.declare_dram_parameter("bq", [DHC, 1], F32, isOutput=False)
    bk_d = nc.declare_dram_parameter("bk", [DHC, 1], F32, isOutput=False)
    bvb_d = nc.declare_dram_parameter("bvb", [128, VW], F32, isOutput=False)
    out_d = nc.declare_dram_parameter("out", [S, D], F32, isOutput=True)

    NQB = 2          # q blocks of 1024
    QB = 1024
    NKT = S // 128   # 16 k tiles

    with TileContext(nc, num_cores=N_CORES) as tc:
        with (
            tc.tile_pool(name="persist", bufs=1) as pp,
            tc.tile_pool(name="pt_pool", bufs=3) as ptp,
            tc.tile_pool(name="rs_pool", bufs=2) as rsp,
            tc.tile_pool(name="ob_pool", bufs=3) as obp,
        ):
            # ---- load inputs ----
            xT = [pp.tile([128, S], BF16, tag=f"xT{i}", name=f"xT{i}") for i in range(4)]
            for i in range(4):
                nc.sync.dma_start(out=xT[i][:], in_=xT_d[128 * i:128 * (i + 1), :])
            wq = [pp.tile([128, DHC], BF16, tag=f"wq{i}", name=f"wq{i}") for i in range(4)]
            wk = [pp.tile([128, DHC], BF16, tag=f"wk{i}", name=f"wk{i}") for i in range(4)]
            wv = [pp.tile([128, VW], BF16, tag=f"wv{i}", name=f"wv{i}") for i in range(4)]
            for i in range(4):
                nc.sync.dma_start(out=wq[i][:], in_=wq_d[128 * i:128 * (i + 1), :])
                nc.sync.dma_start(out=wk[i][:], in_=wk_d[128 * i:128 * (i + 1), :])
                nc.sync.dma_start(out=wv[i][:], in_=wv_d[128 * i:128 * (i + 1), :])
            wo = [pp.tile([128, D], BF16, tag=f"wo{p}", name=f"wo{p}") for p in range(2)]
            for p in range(2):
                nc.sync.dma_start(out=wo[p][:], in_=wo_d[128 * p:128 * (p + 1), :])
            bq = [pp.tile([128, 1], F32, tag=f"bq{p}", name=f"bq{p}") for p in range(2)]
            bk = [pp.tile([128, 1], F32, tag=f"bk{p}", name=f"bk{p}") for p in range(2)]
            for p in range(2):
                nc.sync.dma_start(out=bq[p][:], in_=bq_d[128 * p:128 * (p + 1), :])
                nc.sync.dma_start(out=bk[p][:], in_=bk_d[128 * p:128 * (p + 1), :])
            bvb = pp.tile([128, VW], F32, tag="bvb")
            nc.sync.dma_start(out=bvb[:], in_=bvb_d[:])
            ones = pp.tile([1, HD], F32, tag="ones")
            nc.vector.memset(ones[:], 1.0)

            QT = [pp.tile([128, S], BF16, tag=f"QT{p}", name=f"QT{p}") for p in range(2)]
            KT = [pp.tile([128, S], BF16, tag=f"KT{p}", name=f"KT{p}") for p in range(2)]
            OT = [pp.tile([128, S], BF16, tag=f"OT{p}", name=f"OT{p}") for p in range(2)]
            V = [pp.tile([128, VW], BF16, tag=f"V{st}", name=f"V{st}") for st in range(NKT)]

            # ---- Q / K projections ----
            with tc.tile_pool(name="qk_ps", bufs=2, space="PSUM") as qkps:
                for (w_sb, b_sb, dst) in ((wq, bq, QT), (wk, bk, KT)):
                    for dt in range(2):
                        ps = qkps.tile([128, S], F32, tag="qk")
                        for din in range(4):
                            for st in range(4):
                                nc.tensor.matmul(
                                    ps[:, 512 * st:512 * (st + 1)],
                                    w_sb[din][:, 128 * dt:128 * (dt + 1)],
                                    xT[din][:, 512 * st:512 * (st + 1)],
                                    start=(din == 0), stop=(din == 3),
                                )
                        nc.vector.tensor_scalar(
                            out=dst[dt][:], in0=ps[:], scalar1=b_sb[dt][:],
                            scalar2=None, op0=ALU.add,
                        )

            # ---- V projection (x-tiles stationary, augmented wv moving) ----
            with tc.tile_pool(name="v_ps", bufs=2, space="PSUM") as vps:
                for st in range(NKT):
                    xi, xc = divmod(st, 4)
                    ps = vps.tile([128, VW], F32, tag="v")
                    for din in range(4):
                        nc.tensor.matmul(
                            ps[:],
                            xT[din][:, 128 * st:128 * (st + 1)],
                            wv[din][:],
                            start=(din == 0), stop=(din == 3),
                        )
                    nc.vector.tensor_tensor(
                        out=V[st][:], in0=ps[:], in1=bvb[:], op=ALU.add)

            # ---- attention ----
            with (
                tc.tile_pool(name="s_ps", bufs=2, space="PSUM") as sps,
                tc.tile_pool(name="o_ps", bufs=3, space="PSUM") as ops,
                tc.tile_pool(name="r_ps", bufs=1, space="PSUM") as rps,
            ):
                for h in range(H_CORE):
                    p, m = divmod(h, 2)
                    r0, r1 = 64 * m, 64 * (m + 1)
                    for qb in range(NQB):
                        q0 = QB * qb
                        o_acc = [ops.tile([HD + 1, 512], F32, tag="o", name="o_acc")
                                 for _ in range(2)]

                        def s_mms(kt):
                            stile = sps.tile([128, QB], F32, tag="s")
                            for qt in range(2):
                                nc.tensor.matmul(
                                    stile[:, 512 * qt:512 * (qt + 1)],
                                    KT[p][r0:r1, 128 * kt:128 * (kt + 1)],
                                    QT[p][r0:r1, q0 + 512 * qt:q0 + 512 * (qt + 1)],
                                    start=True, stop=True,
                                )
                            return stile

                        def exp_pv(kt, stile):
                            pt = ptp.tile([128, QB], BF16, tag="pt")
                            nc.scalar.activation(pt[:], stile[:], AF.Exp)
                            for qt in range(2):
                                nc.tensor.matmul(
                                    o_acc[qt][:],
                                    V[kt][:, (HD + 1) * h:(HD + 1) * (h + 1)],
                                    pt[:, 512 * qt:512 * (qt + 1)],
                                    start=(kt == 0), stop=(kt == NKT - 1),
                                )

                        prev = s_mms(0)
                        for kt in range(1, NKT):
                            cur = s_mms(kt)
                            exp_pv(kt - 1, prev)
                            prev = cur
                        exp_pv(NKT - 1, prev)

                        for qt in range(2):
                            rs = rsp.tile([1, 512], F32, tag="rs")
                            nc.vector.reciprocal(rs[:], o_acc[qt][HD:HD + 1, :])
                            rsb = rps.tile([HD, 512], F32, tag="rsb")
                            nc.tensor.matmul(rsb[:], ones[:], rs[:],
                                             start=True, stop=True)
                            # walrus: only one PSUM input per DVE op —
                            # stage the broadcast in SBUF first
                            rsb_sb = rsp.tile([HD, 512], F32, tag="rsb_sb")
                            nc.vector.tensor_copy(rsb_sb[:], rsb[:])
                            nc.vector.tensor_tensor(
                                out=OT[p][r0:r1,
                                          q0 + 512 * qt:q0 + 512 * (qt + 1)],
                                in0=o_acc[qt][0:HD, :], in1=rsb_sb[:],
                                op=ALU.mult,
                            )

            # ---- out projection ----
            with tc.tile_pool(name="op_ps", bufs=2, space="PSUM") as prps:
                for st in range(NKT):
                    ps = prps.tile([128, D], F32, tag="op")
                    for p in range(2):
                        nc.tensor.matmul(
                            ps[:],
                            OT[p][:, 128 * st:128 * (st + 1)],
                            wo[p][:],
                            start=(p == 0), stop=(p == 1),
                        )
                    ob = obp.tile([128, D], F32, tag="ob")
                    nc.vector.tensor_copy(ob[:], ps[:])
                    nc.sync.dma_start(
                        out=out_d[128 * st:128 * (st + 1), :], in_=ob[:])

    nc.compile()
    return nc


def _prep_core(x, wq, bq, wk, bk, wv, bv, wo, bo, b, g):
    hs = slice(DHC * g, DHC * (g + 1))
    xT = np.ascontiguousarray(x[b].T).astype(ml_dtypes.bfloat16)
    wq_c = (wq[:, hs] / 8.0).astype(ml_dtypes.bfloat16)
    bq_c = (bq[hs] / 8.0).astype(np.float32).reshape(DHC, 1)
    wk_c = wk[:, hs].astype(ml_dtypes.bfloat16)
    bk_c = bk[hs].astype(np.float32).reshape(DHC, 1)
    wv_aug = np.zeros((D, VW), np.float32)
    bvb = np.zeros((128, VW), np.float32)
    for h in range(H_CORE):
        c0 = (HD + 1) * h
        wv_aug[:, c0:c0 + HD] = wv[:, DHC * g + HD * h:DHC * g + HD * (h + 1)]
        bvb[:, c0:c0 + HD] = bv[DHC * g + HD * h:DHC * g + HD * (h + 1)][None, :]
        bvb[:, c0 + HD] = 1.0
    wo_c = wo[hs, :].astype(ml_dtypes.bfloat16)
    return {
        "xT": xT,
        "wq": wq_c, "bq": bq_c,
        "wk": wk_c, "bk": bk_c,
        "wv": wv_aug.astype(ml_dtypes.bfloat16), "bvb": bvb,
        "wo": wo_c,
    }


def kernel(x, wq, bq, wk, bk, wv, bv, wo, bo):
    x = np.asarray(x, np.float32)
    wq, bq = np.asarray(wq, np.float32), np.asarray(bq, np.float32)
    wk, bk = np.asarray(wk, np.float32), np.asarray(bk, np.float32)
    wv, bv = np.asarray(wv, np.float32), np.asarray(bv, np.float32)
    wo, bo = np.asarray(wo, np.float32), np.asarray(bo, np.float32)

    if "nc" not in _CACHE:
        _CACHE["nc"] = build_nc()
    nc = _CACHE["nc"]

    in_maps = []
    for c in range(N_CORES):
        b, g = divmod(c, 2)
        in_maps.append(_prep_core(x, wq, bq, wk, bk, wv, bv, wo, bo, b, g))

    res = run_bass_kernel_spmd(nc, in_maps, list(range(N_CORES)))

    out = np.empty((B, S, D), np.float32)
    for b in range(B):
        out[b] = (res.results[2 * b]["out"] + res.results[2 * b + 1]["out"]
                  + bo[None, :])
    return out


# revision 6
# speedup vs baseline: 1.0388x; 1.0388x over previous
"""Multi-head attention (B=4, S=2048, D=512, H=8) on 8 trn2 NeuronCores.

Sharding: core c handles batch b = c//2 and head-group g = c%2 (4 heads,
256 of the 512 model dims). Each core computes its 4 heads' attention and
a partial out-projection [2048, 512]; the host sums the two partials per
batch and adds the output bias.

Device kernel per core (all matmuls bf16 -> f32 PSUM):
  1. QKV projections from pre-transposed xT [512, 2048]:
       Q^T/K^T [128, 2048] per head-pair (wq, bq pre-scaled by 1/8 on host)
       V [128, 260] per seq-tile of 128, with a per-head all-ones column
       (injected via the bias) so the P@V matmul also produces softmax
       row-sums.
  2. Per head, flash-style: S^T tile [128, 1024] = K_h^T . Q_h (K=64),
     exp on ScalarE (PSUM -> SBUF bf16, double-buffered, software
     pipelined), P^T accumulated into O^T [65, 512] over 16 k-tiles.
  3. Normalize: reciprocal of the row-sum row, broadcast via a K=1 ones
     matmul, multiply into the stacked O^T [128, 2048] per pair.
  4. Out-projection: out^T partial = sum_p OT[p]^T(chunk) @ wo[p].
No max-subtraction in softmax: scores are O(1) by construction, exp is
safe, and the reference softmax is shift-invariant.
"""

import numpy as np
import ml_dtypes

import concourse.bacc as bacc
import concourse.mybir as mybir
from concourse.tile import TileContext
from concourse.bass_utils import run_bass_kernel_spmd

BF16 = mybir.dt.bfloat16
F32 = mybir.dt.float32
AF = mybir.ActivationFunctionType
ALU = mybir.AluOpType

B, S, D = 4, 2048, 512
H_CORE, HD = 4, 64          # heads per core, head dim
DHC = H_CORE * HD           # 256 dims per core
VW = H_CORE * (HD + 1)      # 260: V augmented with one ones-column per head
N_CORES = 8

_CACHE = {}


def build_nc():
    nc = bacc.Bacc("TRN2", target_bir_lowering=False, debug=False,
                   num_devices=N_CORES)

    xT_d = nc.declare_dram_parameter("xT", [D, S], BF16, isOutput=False)
    wq_d = nc.declare_dram_parameter("wq", [D, DHC], BF16, isOutput=False)
    wk_d = nc.declare_dram_parameter("wk", [D, DHC], BF16, isOutput=False)
    wv_d = nc.declare_dram_parameter("wv", [D, VW], BF16, isOutput=False)
    wo_d = nc.declare_dram_parameter("wo", [DHC, D], BF16, isOutput=False)
    bq_d = nc.declare_dram_parameter("bq", [DHC, 1], F32, isOutput=False)
    bk_d = nc.declare_dram_parameter("bk", [DHC, 1], F32, isOutput=False)
    bvb_d = nc.declare_dram_parameter("bvb", [128, VW], F32, isOutput=False)
    out_d = nc.declare_dram_parameter("out", [S, D], F32, isOutput=True)

    NQB = 2          # q blocks of 1024
    QB = 1024
    NKT = S // 128   # 16 k tiles

    with TileContext(nc, num_cores=N_CORES) as tc:
        with (
            tc.tile_pool(name="persist", bufs=1) as pp,
            tc.tile_pool(name="pt_pool", bufs=3) as ptp,
            tc.tile_pool(name="rs_pool", bufs=2) as rsp,
            tc.tile_pool(name="ob_pool", bufs=3) as obp,
        ):
            # ---- load inputs (Q-proj operands first so PE starts ASAP) ----
            xT = [pp.tile([128, S], BF16, tag=f"xT{i}", name=f"xT{i}") for i in range(4)]
            wq = [pp.tile([128, DHC], BF16, tag=f"wq{i}", name=f"wq{i}") for i in range(4)]
            wk = [pp.tile([128, DHC], BF16, tag=f"wk{i}", name=f"wk{i}") for i in range(4)]
            wv = [pp.tile([128, VW], BF16, tag=f"wv{i}", name=f"wv{i}") for i in range(4)]
            bq = [pp.tile([128, 1], F32, tag=f"bq{p}", name=f"bq{p}") for p in range(2)]
            bk = [pp.tile([128, 1], F32, tag=f"bk{p}", name=f"bk{p}") for p in range(2)]
            for i in range(4):
                nc.sync.dma_start(out=xT[i][:], in_=xT_d[128 * i:128 * (i + 1), :])
                nc.sync.dma_start(out=wq[i][:], in_=wq_d[128 * i:128 * (i + 1), :])
            for p in range(2):
                nc.sync.dma_start(out=bq[p][:], in_=bq_d[128 * p:128 * (p + 1), :])
            for i in range(4):
                nc.sync.dma_start(out=wk[i][:], in_=wk_d[128 * i:128 * (i + 1), :])
            for p in range(2):
                nc.sync.dma_start(out=bk[p][:], in_=bk_d[128 * p:128 * (p + 1), :])
            for i in range(4):
                nc.sync.dma_start(out=wv[i][:], in_=wv_d[128 * i:128 * (i + 1), :])
            bvb = pp.tile([128, VW], F32, tag="bvb")
            nc.sync.dma_start(out=bvb[:], in_=bvb_d[:])
            wo = [pp.tile([128, D], BF16, tag=f"wo{p}", name=f"wo{p}") for p in range(2)]
            for p in range(2):
                nc.sync.dma_start(out=wo[p][:], in_=wo_d[128 * p:128 * (p + 1), :])
            QT = [pp.tile([128, S], BF16, tag=f"QT{p}", name=f"QT{p}") for p in range(2)]
            KT = [pp.tile([128, S], BF16, tag=f"KT{p}", name=f"KT{p}") for p in range(2)]
            OT = [pp.tile([128, S], BF16, tag=f"OT{p}", name=f"OT{p}") for p in range(2)]
            V = [pp.tile([128, VW], BF16, tag=f"V{st}", name=f"V{st}") for st in range(NKT)]

            # ---- Q / K projections ----
            with tc.tile_pool(name="qk_ps", bufs=2, space="PSUM") as qkps:
                for (w_sb, b_sb, dst) in ((wq, bq, QT), (wk, bk, KT)):
                    for dt in range(2):
                        ps = qkps.tile([128, S], F32, tag="qk")
                        for din in range(4):
                            for st in range(4):
                                nc.tensor.matmul(
                                    ps[:, 512 * st:512 * (st + 1)],
                                    w_sb[din][:, 128 * dt:128 * (dt + 1)],
                                    xT[din][:, 512 * st:512 * (st + 1)],
                                    start=(din == 0), stop=(din == 3),
                                )
                        nc.vector.tensor_scalar(
                            out=dst[dt][:], in0=ps[:], scalar1=b_sb[dt][:],
                            scalar2=None, op0=ALU.add,
                        )

            # ---- V projection (x-tiles stationary, augmented wv moving) ----
            with tc.tile_pool(name="v_ps", bufs=2, space="PSUM") as vps:
                for st in range(NKT):
                    xi, xc = divmod(st, 4)
                    ps = vps.tile([128, VW], F32, tag="v")
                    for din in range(4):
                        nc.tensor.matmul(
                            ps[:],
                            xT[din][:, 128 * st:128 * (st + 1)],
                            wv[din][:],
                            start=(din == 0), stop=(din == 3),
                        )
                    nc.vector.tensor_tensor(
                        out=V[st][:], in0=ps[:], in1=bvb[:], op=ALU.add)

            # ---- attention ----
            with (
                tc.tile_pool(name="s_ps", bufs=2, space="PSUM") as sps,
                tc.tile_pool(name="o_ps", bufs=4, space="PSUM") as ops,
            ):
                for h in range(H_CORE):
                    p, m = divmod(h, 2)
                    r0, r1 = 64 * m, 64 * (m + 1)
                    for qb in range(NQB):
                        q0 = QB * qb
                        o_acc = [ops.tile([HD + 1, 512], F32, tag="o", name="o_acc")
                                 for _ in range(2)]

                        def s_mms(kt):
                            stile = sps.tile([128, QB], F32, tag="s")
                            for qt in range(2):
                                nc.tensor.matmul(
                                    stile[:, 512 * qt:512 * (qt + 1)],
                                    KT[p][r0:r1, 128 * kt:128 * (kt + 1)],
                                    QT[p][r0:r1, q0 + 512 * qt:q0 + 512 * (qt + 1)],
                                    start=True, stop=True,
                                )
                            return stile

                        def exp_pv(kt, stile):
                            pt = ptp.tile([128, QB], BF16, tag="pt")
                            nc.scalar.activation(pt[:], stile[:], AF.Exp)
                            for qt in range(2):
                                nc.tensor.matmul(
                                    o_acc[qt][:],
                                    V[kt][:, (HD + 1) * h:(HD + 1) * (h + 1)],
                                    pt[:, 512 * qt:512 * (qt + 1)],
                                    start=(kt == 0), stop=(kt == NKT - 1),
                                )

                        prev = s_mms(0)
                        for kt in range(1, NKT):
                            cur = s_mms(kt)
                            exp_pv(kt - 1, prev)
                            prev = cur
                        exp_pv(NKT - 1, prev)

                        # epilogue stays off the PE/ACT critical path:
                        # DVE reciprocal + gpsimd broadcast + DVE multiply
                        for qt in range(2):
                            rec = rsp.tile([1, 512], F32, tag="rec",
                                           name="rec")
                            nc.vector.reciprocal(
                                rec[:], o_acc[qt][HD:HD + 1, :])
                            recB = rsp.tile([HD, 512], F32, tag="recB",
                                            name="recB")
                            nc.gpsimd.partition_broadcast(recB[:], rec[:])
                            nc.vector.tensor_tensor(
                                out=OT[p][r0:r1,
                                          q0 + 512 * qt:q0 + 512 * (qt + 1)],
                                in0=o_acc[qt][0:HD, :], in1=recB[:],
                                op=ALU.mult,
                            )

            # ---- out projection ----
            with tc.tile_pool(name="op_ps", bufs=2, space="PSUM") as prps:
                for st in range(NKT):
                    ps = prps.tile([128, D], F32, tag="op")
                    for p in range(2):
                        nc.tensor.matmul(
                            ps[:],
                            OT[p][:, 128 * st:128 * (st + 1)],
                            wo[p][:],
                            start=(p == 0), stop=(p == 1),
                        )
                    ob = obp.tile([128, D], F32, tag="ob")
                    nc.vector.tensor_copy(ob[:], ps[:])
                    nc.sync.dma_start(
                        out=out_d[128 * st:128 * (st + 1), :], in_=ob[:])

    nc.compile()
    return nc


def _prep_core(x, wq, bq, wk, bk, wv, bv, wo, bo, b, g):
    hs = slice(DHC * g, DHC * (g + 1))
    xT = np.ascontiguousarray(x[b].T).astype(ml_dtypes.bfloat16)
    wq_c = (wq[:, hs] / 8.0).astype(ml_dtypes.bfloat16)
    bq_c = (bq[hs] / 8.0).astype(np.float32).reshape(DHC, 1)
    wk_c = wk[:, hs].astype(ml_dtypes.bfloat16)
    bk_c = bk[hs].astype(np.float32).reshape(DHC, 1)
    wv_aug = np.zeros((D, VW), np.float32)
    bvb = np.zeros((128, VW), np.float32)
    for h in range(H_CORE):
        c0 = (HD + 1) * h
        wv_aug[:, c0:c0 + HD] = wv[:, DHC * g + HD * h:DHC * g + HD * (h + 1)]
        bvb[:, c0:c0 + HD] = bv[DHC * g + HD * h:DHC * g + HD * (h + 1)][None, :]
        bvb[:, c0 + HD] = 1.0
    wo_c = wo[hs, :].astype(ml_dtypes.bfloat16)
    return {
        "xT": xT,
        "wq": wq_c, "bq": bq_c,
        "wk": wk_c, "bk": bk_c,
        "wv": wv_aug.astype(ml_dtypes.bfloat16), "bvb": bvb,
        "wo": wo_c,
    }


def kernel(x, wq, bq, wk, bk, wv, bv, wo, bo):
    x = np.asarray(x, np.float32)
    wq, bq = np.asarray(wq, np.float32), np.asarray(bq, np.float32)
    wk, bk = np.asarray(wk, np.float32), np.asarray(bk, np.float32)
    wv, bv = np.asarray(wv, np.float32), np.asarray(bv, np.float32)
    wo, bo = np.asarray(wo, np.float32), np.asarray(bo, np.float32)

    if "nc" not in _CACHE:
        _CACHE["nc"] = build_nc()
    nc = _CACHE["nc"]

    in_maps = []
    for c in range(N_CORES):
        b, g = divmod(c, 2)
        in_maps.append(_prep_core(x, wq, bq, wk, bk, wv, bv, wo, bo, b, g))

    res = run_bass_kernel_spmd(nc, in_maps, list(range(N_CORES)))

    out = np.empty((B, S, D), np.float32)
    for b in range(B):
        out[b] = (res.results[2 * b]["out"] + res.results[2 * b + 1]["out"]
                  + bo[None, :])
    return out
